# revision 8
# baseline (speedup 1.0000x reference)
"""Trainium2 Bass kernel for the autoregressive GRU decode head.

Problem: context = mean over zones of encoded_features[:, -1]  -> (B, D)
then 12 autoregressive steps of a 2-layer GRU (H=256) + linear projection
to N=256 zones.  B=1024, data-parallel across 8 NeuronCores (128 batch each).

Structure (per core, feature-major / "transposed" activations):
  actT (128p, 4 slots, 128) bf16 : [h0 c0, h0 c1, h1 c0, h1 c1]
  Gate tensors (PSUM) use layout [p, c*128 + b]; r and z halves of the
  rz gates live in SEPARATE PSUM tiles so the r-sigmoid can fire as soon
  as the r matmuls complete.
  ALL gate biases (incl. the per-step folded step-embedding contribution)
  are injected as K=1 bias-row matmuls into the PSUM accumulation groups,
  so every elementwise chain op is a single full-width instruction:
    sig_r(256) -> t = ghn*r (TT) -> v = gin+t (TT) -> tanh -> q=(z-1)*n
    (STT, all-bf16 SBUF for DVE fast mode) -> hv = c - q  with c = z*h
    computed off the critical path.
  The prediction feedback is folded into layer-0 weights (W_pred@W_out).
  Matmuls for step t+1 that depend only on h0' run during chain1 of step
  t; junk matmul fillers keep the PE p-state at max (the PE halves its
  clock unless continuously busy ~3us).
The encoded_features slice is streamed as bf16 (host-converted); the zone
mean is a pairwise TT-add tree on DVE (bf16 2x levels, f32 tail).
"""

import sys

for _p in ("/opt/trn_rl_repo",):
    if _p not in sys.path:
        sys.path.insert(0, _p)

import numpy as np
import ml_dtypes

import concourse.bass as bass
import concourse.tile as tile
from concourse import mybir
from concourse.vector_clock import ScopedClock

BF16 = ml_dtypes.bfloat16

B, T, NZ, D = 1024, 8, 256, 256
H = 256
STEPS = 12
N_CORES = 8
PC = B // N_CORES  # 128 batch per core

F32 = mybir.dt.float32
BF = mybir.dt.bfloat16
AF = mybir.ActivationFunctionType
OP = mybir.AluOpType

# bias-row slot layout (each slot is 128 wide) in the (1, NBROW*128) bf16
# bias-rows tensor.
_R0R = 0                       # per step: rz0 r rows (2 each)
_R0Z = _R0R + STEPS * 2        # per step: rz0 z rows
_IN0 = _R0Z + STEPS * 2        # per step: in0 rows
_HN0 = _IN0 + STEPS * 2        # 2
_R1R = _HN0 + 2                # 2
_R1Z = _R1R + 2                # 2
_IN1 = _R1Z + 2                # 2
_HN1 = _IN1 + 2                # 2
NBROW = _HN1 + 2

# phase-1 tail warm-up fillers (~213ns each), gated on the 5th enc chunk
NWARM = 12


def _install_tile_drain_patch():
    """walrus (CoreV3) rejects >1 sync wait on the tail drain; spill extras
    onto preceding sync nops."""
    if getattr(tile.TileContext, "_drain_patch_installed", False):
        return

    def _patched(self, tick_clock, wait_clock):
        nc = self.nc
        bb = nc.cur_bb.bb
        drain_bi = nc.sync.drain()
        drain_inst = drain_bi.ins
        wait_clock.add_sem_waits(
            drain_inst, ScopedClock({None: tick_clock.global_clock})
        )
        w = drain_inst.sync_info.on_wait if drain_inst.sync_info else None
        maxw = 1
        if w and len(w) > maxw:
            extra = list(w[maxw:])
            drain_inst.sync_info.on_wait = list(w[:maxw])
            idx = bb.instructions.index(drain_inst)
            nops = []
            for i in range(0, len(extra), maxw):
                nop_bi = nc.sync.nop()
                nop = nop_bi.ins
                si = nop.sync_info
                nop.sync_info = mybir.SyncInfo(
                    on_wait=extra[i : i + maxw],
                    on_update=(si.on_update if si else []),
                )
                bb.instructions.remove(nop)
                nops.append(nop)
            bb.instructions[idx:idx] = nops
        nc.all_engine_barrier()
        popped = nc._tile_sem_poison_stack.pop()
        assert popped is self._sem_poison
        nc.clear_and_free_semaphores(list(self.sems.allocated().values()))
        nc.all_engine_barrier()

    tile.TileContext._drain_and_barrier = _patched
    tile.TileContext._drain_patch_installed = True


def _split_waits(nc, maxw=1):
    """This walrus build rejects instructions carrying more than ~1 sem
    wait; spill extra waits onto same-engine nops placed just before."""
    for bb in nc.main_func.blocks:
        new_list = []
        for inst in bb.instructions:
            si = inst.sync_info
            w = list(si.on_wait) if si and si.on_wait else []
            if len(w) > maxw:
                keep = w[len(w) - maxw:]
                extra = w[: len(w) - maxw]
                si.on_wait = keep
                for i in range(0, len(extra), maxw):
                    nop = mybir.InstNoOp(
                        name=f"{inst.name}-sw{i}", ins=[], outs=[]
                    )
                    nop.engine = inst.engine
                    nop.sync_info = mybir.SyncInfo(
                        on_wait=extra[i : i + maxw], on_update=[]
                    )
                    nc.register_instruction(nop)
                    new_list.append(nop)
            new_list.append(inst)
        bb.instructions[:] = new_list


class _Group:
    """Tracks start/stop flags for a PSUM accumulation group whose matmuls
    are emitted in several program-order batches."""

    def __init__(self, total):
        self.total = total
        self.emitted = 0

    def flags(self):
        start = self.emitted == 0
        self.emitted += 1
        return start, self.emitted == self.total


def build_kernel(nsteps=12):
    """Build the per-core Bass graph (SPMD: same graph on all 8 cores)."""
    _install_tile_drain_patch()
    nc = bass.Bass()

    enc = nc.declare_dram_parameter("enc", [PC, NZ, D], BF, isOutput=False)
    wrz0 = nc.declare_dram_parameter("wrz0", [4, 128, 512], BF, isOutput=False)
    win0 = nc.declare_dram_parameter("win0", [2, 128, 256], BF, isOutput=False)
    whn0 = nc.declare_dram_parameter("whn0", [2, 128, 256], BF, isOutput=False)
    wrz1 = nc.declare_dram_parameter("wrz1", [4, 128, 512], BF, isOutput=False)
    win1 = nc.declare_dram_parameter("win1", [2, 128, 256], BF, isOutput=False)
    whn1 = nc.declare_dram_parameter("whn1", [2, 128, 256], BF, isOutput=False)
    wout = nc.declare_dram_parameter("wout", [2, 128, 256], BF, isOutput=False)
    brows = nc.declare_dram_parameter("brows", [1, NBROW * 128], BF,
                                      isOutput=False)
    onesr = nc.declare_dram_parameter("onesr", [1, 128], BF, isOutput=False)
    ident = nc.declare_dram_parameter("ident", [128, 128], F32, isOutput=False)
    boutb = nc.declare_dram_parameter("boutb", [128, 256], F32, isOutput=False)
    out = nc.declare_dram_parameter("out", [PC, STEPS, NZ], F32, isOutput=True)

    with tile.TileContext(nc) as tc:
        with (
            tc.tile_pool(name="consts", bufs=1) as consts,
            tc.tile_pool(name="state", bufs=1) as state,
            tc.tile_pool(name="enc_pool", bufs=4) as enc_pool,
            tc.tile_pool(name="gates", bufs=2) as gates,
            tc.tile_pool(name="ostage", bufs=2) as ostage,
            tc.tile_pool(name="psum", bufs=1, space="PSUM") as psum,
        ):
            # ---- enc chunk DMAs first: first useful work is the stream ----
            ZCHS = [64, 64, 64, 32, 16, 16]
            NCH = len(ZCHS)
            e_tiles = []
            z0 = 0
            for zch in ZCHS:
                e_sb = enc_pool.tile([128, 64 * D], BF, tag="echunk")
                nc.sync.dma_start(
                    e_sb[:, : zch * D], enc[:, z0 : z0 + zch, :]
                )
                e_tiles.append(e_sb)
                z0 += zch

            # ---- small consts + weights on the scalar HWDGE queue ----
            ones_row = consts.tile([1, 128], BF, tag="ones")
            nc.scalar.dma_start(ones_row[:], onesr[:])
            brow_sb = consts.tile([1, NBROW * 128], BF, tag="brow")
            nc.scalar.dma_start(brow_sb[:], brows[:])
            identity = consts.tile([128, 128], F32, tag="ident")
            nc.scalar.dma_start(identity[:], ident[:])
            w_sb = {}
            for name, ap in (
                ("wrz0", wrz0), ("win0", win0), ("whn0", whn0),
                ("wrz1", wrz1), ("win1", win1), ("whn1", whn1),
                ("wout", wout),
            ):
                kc = ap.shape[0]
                t_ = consts.tile([128, kc, ap.shape[2]], BF, tag=name)
                nc.scalar.dma_start(t_[:], ap.rearrange("k p m -> p k m"))
                w_sb[name] = t_
            boutb_sb = consts.tile([128, 256], F32, tag="boutb")
            nc.scalar.dma_start(boutb_sb[:], boutb[:])

            # prewarm the sigmoid/tanh ACT table during the stream
            warm = consts.tile([128, 1], F32, tag="warm")
            nc.scalar.activation(warm[:], identity[:, 0:1], AF.Sigmoid)

            # ---- phase 1: zone-mean tree per chunk on DVE ----
            partials = state.tile([128, NCH, D], F32, tag="partials")
            for i in range(NCH):
                e_sb = e_tiles[i]
                w = ZCHS[i] * D
                while w > 8 * D:
                    h = w // 2
                    nc.vector.tensor_tensor(
                        e_sb[:, 0:h], e_sb[:, 0:h], e_sb[:, h:w], OP.add
                    )
                    w = h
                # f32 tail: GpSimd for the early chunks (it has slack there);
                # DVE for the late chunks + all combines (shortest exit latency)
                eng = nc.gpsimd if i < 3 else nc.vector
                tmp = gates.tile([128, 4 * D], F32, tag="redtail")
                eng.tensor_tensor(
                    tmp[:], e_sb[:, 0 : 4 * D], e_sb[:, 4 * D : 8 * D], OP.add
                )
                eng.tensor_tensor(
                    tmp[:, 0 : 2 * D], tmp[:, 0 : 2 * D], tmp[:, 2 * D : 4 * D],
                    OP.add,
                )
                eng.tensor_tensor(
                    partials[:, i, :], tmp[:, 0:D], tmp[:, D : 2 * D], OP.add
                )
                if i:
                    nc.vector.tensor_tensor(
                        partials[:, 0, :], partials[:, 0, :], partials[:, i, :],
                        OP.add,
                    )
            ctx = partials[:, 0, :]
            nc.vector.tensor_scalar_mul(ctx, ctx, 1.0 / NZ)

            # ---- PE fillers: dependency-free junk matmuls ----
            junk = psum.tile([128, 512], F32, tag="junk", bufs=1)
            dmy_lhs = w_sb["wrz0"][:, 1, 0:128]
            dmy_rhs = w_sb["wrz0"][:, 2, :]

            def dummies(n):
                for _ in range(n):
                    nc.tensor.matmul(junk[:], dmy_lhs, dmy_rhs,
                                     start=True, stop=True)

            def gated_dummies(s_, v_, n_):
                # K=1 fillers (no weight-load cost) whose deps track the
                # chain ops, so they drain in lockstep with the chain and
                # never pile up in front of the next gating matmuls
                row = brow_sb[0:1, 0:128]
                for _ in range(3):
                    nc.tensor.matmul(junk[:, 0:256], row, s_[0:1, 0:256],
                                     start=True, stop=True)
                for _ in range(2):
                    nc.tensor.matmul(junk[:, 0:256], row, s_[0:1, 256:512],
                                     start=True, stop=True)
                for _ in range(2):
                    nc.tensor.matmul(junk[:, 0:256], identity[0:1, :],
                                     v_[0:1, 0:256], start=True, stop=True)
                for _ in range(2):
                    nc.tensor.matmul(junk[:, 0:256], row, n_[0:1, 0:256],
                                     start=True, stop=True)

            def warm_dummies(n, gate_tile):
                # gated on a late enc chunk: ramps the PE p-state up just
                # before the decode phase begins
                for _ in range(n):
                    nc.tensor.matmul(junk[:], dmy_lhs, gate_tile[:, 0:512],
                                     start=True, stop=True)

            warm_dummies(NWARM, e_tiles[4])

            # ---- state: actT slots [h0c0, h0c1, h1c0, h1c1] ----
            # ctx already carries the 1/NZ mean scale
            actT = state.tile([128, 4, 128], BF, tag="actT")
            for c in range(2):
                ctps_t = psum.tile([128, 256], F32, tag="predB", bufs=1)
                ctps = ctps_t[:, 0:128]
                nc.tensor.transpose(
                    ctps[:], ctx[:, c * 128 : (c + 1) * 128], identity[:]
                )
                nc.scalar.activation(actT[:, c, :], ctps[:], AF.Copy)
                nc.vector.tensor_copy(actT[:, 2 + c, :], ctps[:])
            dummies(5)

            def brow_mm(g, grp, slot, c):
                st, sp = grp.flags()
                nc.tensor.matmul(
                    g[:, c * 128 : (c + 1) * 128],
                    brow_sb[0:1, (slot + c) * 128 : (slot + c + 1) * 128],
                    ones_row[:], start=st, stop=sp,
                )

            def gate_mms(g, grp, w_t, kis, slots, mbase, nch):
                for m in range(nch):
                    ms = slice((mbase + m) * 128, (mbase + m + 1) * 128)
                    gs = slice(m * 128, (m + 1) * 128)
                    for ki, slot in zip(kis, slots):
                        st, sp = grp.flags()
                        nc.tensor.matmul(
                            g[:, gs], w_t[:, ki, ms], actT[:, slot, :],
                            start=st, stop=sp,
                        )

            def chain(layer, g_r, g_z, g_ni):
                h_lo = 0 if layer == 0 else 2
                g_hn = g_ni[:, 0:256]
                g_in = g_ni[:, 256:512]
                s_ = gates.tile([128, 512], BF, tag=f"S{layer}")
                nc.scalar.activation(s_[:, 0:256], g_r[:], AF.Sigmoid)
                nc.scalar.activation(s_[:, 256:512], g_z[:], AF.Sigmoid)
                # s2 = 1 - z, computed exactly as sigmoid(-gz) off-path
                s2 = gates.tile([128, 256], BF, tag=f"S2{layer}")
                nc.scalar.activation(s2[:], g_z[:], AF.Sigmoid, scale=-1.0)
                sr = s_[:, 0:256]
                sz = s_[:, 256:512]
                t_ = gates.tile([128, 256], F32, tag=f"tt{layer}")
                nc.vector.tensor_tensor(t_[:], g_hn, sr, OP.mult)
                v_ = gates.tile([128, 256], F32, tag=f"vv{layer}")
                nc.vector.tensor_tensor(v_[:], g_in, t_[:], OP.add)
                hv = actT[:, h_lo : h_lo + 2, :].rearrange("p a b -> p (a b)")
                c_ = gates.tile([128, 256], BF, tag=f"cc{layer}")
                nc.vector.tensor_tensor(c_[:], sz, hv, OP.mult)
                n_ = gates.tile([128, 256], BF, tag=f"nn{layer}")
                nc.scalar.activation(n_[:], v_[:], AF.Tanh)
                q_ = gates.tile([128, 256], BF, tag=f"qq{layer}")
                nc.vector.tensor_tensor(q_[:], s2[:], n_[:], OP.mult)
                nc.vector.tensor_tensor(hv, c_[:], q_[:], OP.add)
                return s_, v_, n_

            # ---- phase 2: decode steps ----
            # W1(0): step-0 rz0/hn0/in0 h0-part + bias rows (gated on actT)
            g_r0 = psum.tile([128, 256], F32, tag="r0", bufs=1)
            g_z0 = psum.tile([128, 256], F32, tag="z0", bufs=1)
            g_ni0 = psum.tile([128, 512], F32, tag="ni0", bufs=1)
            g_hn0 = g_ni0[:, 0:256]
            g_in0 = g_ni0[:, 256:512]
            gr0 = _Group(6)              # step 0: no feedback
            gz0 = _Group(6)
            gni0 = _Group(8)
            for c in range(2):
                brow_mm(g_r0, gr0, _R0R, c)
            gate_mms(g_r0, gr0, w_sb["wrz0"], (2, 3), (0, 1), 0, 2)
            for c in range(2):
                brow_mm(g_z0, gz0, _R0Z, c)
            gate_mms(g_z0, gz0, w_sb["wrz0"], (2, 3), (0, 1), 2, 2)
            for c in range(2):
                brow_mm(g_hn0, gni0, _HN0, c)
            gate_mms(g_hn0, gni0, w_sb["whn0"], (0, 1), (0, 1), 0, 2)
            for c in range(2):
                brow_mm(g_in0, gni0, _IN0, c)

            prev_pb = None
            for t in range(nsteps):
                # --- block1: pred-feedback matmuls (need h1 of t-1) ---
                if t > 0:
                    gate_mms(g_r0, gr0, w_sb["wrz0"], (0, 1), (2, 3), 0, 2)
                    gate_mms(g_z0, gz0, w_sb["wrz0"], (0, 1), (2, 3), 2, 2)
                    gate_mms(g_in0, gni0, w_sb["win0"], (0, 1), (2, 3), 0, 2)

                # --- chain0 ---
                ch0 = chain(0, g_r0, g_z0, g_ni0)

                # --- W2 window (runs on PE during chain0) ---
                g_r1 = psum.tile([128, 256], F32, tag="r1", bufs=1)
                g_z1 = psum.tile([128, 256], F32, tag="z1", bufs=1)
                g_ni1 = psum.tile([128, 512], F32, tag="ni1", bufs=1)
                g_hn1 = g_ni1[:, 0:256]
                g_in1 = g_ni1[:, 256:512]
                gr1 = _Group(10)
                gz1 = _Group(10)
                gni1 = _Group(12)
                for c in range(2):
                    brow_mm(g_r1, gr1, _R1R, c)
                gate_mms(g_r1, gr1, w_sb["wrz1"], (2, 3), (2, 3), 0, 2)
                for c in range(2):
                    brow_mm(g_z1, gz1, _R1Z, c)
                gate_mms(g_z1, gz1, w_sb["wrz1"], (2, 3), (2, 3), 2, 2)
                for c in range(2):
                    brow_mm(g_hn1, gni1, _HN1, c)
                gate_mms(g_hn1, gni1, w_sb["whn1"], (0, 1), (2, 3), 0, 2)
                for c in range(2):
                    brow_mm(g_in1, gni1, _IN1, c)
                # out projection of t-1 (batch-major, off the recurrence)
                if prev_pb is not None:
                    g_pb, pt = prev_pb
                    o_ = ostage.tile([128, 256], F32, tag="ost")
                    nc.vector.tensor_tensor(o_[:], g_pb[:], boutb_sb[:], OP.add)
                    nc.sync.dma_start(out[:, pt, :], o_[:])
                    prev_pb = None
                gated_dummies(*ch0)

                # --- block2: layer-1 input-part matmuls (need h0 of t) ---
                gate_mms(g_r1, gr1, w_sb["wrz1"], (0, 1), (0, 1), 0, 2)
                gate_mms(g_z1, gz1, w_sb["wrz1"], (0, 1), (0, 1), 2, 2)
                gate_mms(g_in1, gni1, w_sb["win1"], (0, 1), (0, 1), 0, 2)

                # --- chain1 ---
                ch1 = chain(1, g_r1, g_z1, g_ni1)

                # --- W1(t+1) window (runs on PE during chain1) ---
                if t + 1 < nsteps:
                    tn = t + 1
                    g_r0 = psum.tile([128, 256], F32, tag="r0", bufs=1)
                    g_z0 = psum.tile([128, 256], F32, tag="z0", bufs=1)
                    g_ni0 = psum.tile([128, 512], F32, tag="ni0", bufs=1)
                    g_hn0 = g_ni0[:, 0:256]
                    g_in0 = g_ni0[:, 256:512]
                    gr0 = _Group(10)
                    gz0 = _Group(10)
                    gni0 = _Group(12)
                    for c in range(2):
                        brow_mm(g_r0, gr0, _R0R + tn * 2, c)
                    gate_mms(g_r0, gr0, w_sb["wrz0"], (2, 3), (0, 1), 0, 2)
                    for c in range(2):
                        brow_mm(g_z0, gz0, _R0Z + tn * 2, c)
                    gate_mms(g_z0, gz0, w_sb["wrz0"], (2, 3), (0, 1), 2, 2)
                    for c in range(2):
                        brow_mm(g_hn0, gni0, _HN0, c)
                    gate_mms(g_hn0, gni0, w_sb["whn0"], (0, 1), (0, 1), 0, 2)
                    for c in range(2):
                        brow_mm(g_in0, gni0, _IN0 + tn * 2, c)
                # prediction of step t (reads h1' just produced)
                g_pb = psum.tile([128, 256], F32, tag="predB", bufs=1)
                for ki, slot in ((0, 2), (1, 3)):
                    nc.tensor.matmul(
                        g_pb[:], actT[:, slot, :], w_sb["wout"][:, ki, :],
                        start=(ki == 0), stop=(ki == 1),
                    )
                prev_pb = (g_pb, t)
                if t + 1 < nsteps:
                    gated_dummies(*ch1)

            # final output write
            g_pb, pt = prev_pb
            o_ = ostage.tile([128, 256], F32, tag="ost")
            nc.vector.tensor_tensor(o_[:], g_pb[:], boutb_sb[:], OP.add)
            nc.sync.dma_start(out[:, pt, :], o_[:])

    _split_waits(nc)
    return nc


def _prep_inputs(encoded_features, step_emb, W_ih0, W_hh0, b_ih0, b_hh0,
                 W_ih1, W_hh1, b_ih1, b_hh1, W_out, b_out):
    """Host-side: slice/shard the big input, transpose + cast weights,
    fold the output projection into layer-0 input weights, fold the
    step-embedding matmul + all additive constants into bias rows."""
    f4 = np.float32
    enc_last = np.asarray(encoded_features)[:, -1].astype(BF16)
    enc_last = np.ascontiguousarray(enc_last)

    W_ih0 = np.asarray(W_ih0, f4)
    W_hh0 = np.asarray(W_hh0, f4)
    W_ih1 = np.asarray(W_ih1, f4)
    W_hh1 = np.asarray(W_hh1, f4)
    W_out = np.asarray(W_out, f4)
    step_emb = np.asarray(step_emb, f4)
    b_ih0 = np.asarray(b_ih0, f4)
    b_hh0 = np.asarray(b_hh0, f4)
    b_ih1 = np.asarray(b_ih1, f4)
    b_hh1 = np.asarray(b_hh1, f4)
    b_out = np.asarray(b_out, f4)

    W_emb = W_ih0[:, :D]          # (768, 256)
    W_pred = W_ih0[:, D:]         # (768, 256)
    W_fold = W_pred @ W_out       # (768, 256): pred feedback folded onto h1
    b_fold = W_pred @ b_out       # (768,)

    # gi_emb[t] = W_emb @ step_emb[t] + b_ih0  -> (12, 768)
    gi_emb = step_emb[:STEPS] @ W_emb.T + b_ih0[None, :]

    def kstack(*mats_cols):
        chunks = []
        for mat, cols in mats_cols:
            mt = np.ascontiguousarray(mat.T[:, cols])  # (K, M)
            for k in range(0, mt.shape[0], 128):
                chunks.append(mt[k : k + 128])
        return np.stack(chunks).astype(BF16)  # (nk, 128, M)

    rz = slice(0, 512)
    ng = slice(512, 768)
    wrz0 = kstack((W_fold, rz), (W_hh0, rz))          # K: h1c0,h1c1,h0c0,h0c1
    win0 = kstack((W_fold, ng))
    whn0 = kstack((W_hh0, ng))
    wrz1 = kstack((W_ih1, rz), (W_hh1, rz))           # K: h0c0,h0c1,h1c0,h1c1
    win1 = kstack((W_ih1, ng))
    whn1 = kstack((W_hh1, ng))
    wout = np.stack([np.ascontiguousarray(W_out.T)[k : k + 128] for k in (0, 128)]
                    ).astype(BF16)                    # (2, 128, 256)

    brows = np.zeros(NBROW * 128, f4)

    def put(base, vec):
        brows[base * 128 : base * 128 + len(vec)] = vec

    for t in range(STEPS):
        extra = b_fold if t > 0 else np.zeros_like(b_fold)
        rzrow = gi_emb[t, :512] + b_hh0[:512] + extra[:512]
        put(_R0R + t * 2, rzrow[:256])
        put(_R0Z + t * 2, rzrow[256:])
        put(_IN0 + t * 2, gi_emb[t, 512:] + extra[512:])
    put(_HN0, b_hh0[512:])
    rz1row = b_ih1[:512] + b_hh1[:512]
    put(_R1R, rz1row[:256])
    put(_R1Z, rz1row[256:])
    put(_IN1, b_ih1[512:])
    put(_HN1, b_hh1[512:])
    brows = brows.astype(BF16)[None, :]

    onesr = np.ones((1, 128), BF16)
    ident = np.eye(128, dtype=f4)
    boutb = np.broadcast_to(b_out[None, :], (128, 256)).astype(f4).copy()

    shared = dict(wrz0=wrz0, win0=win0, whn0=whn0, wrz1=wrz1, win1=win1,
                  whn1=whn1, wout=wout, brows=brows, onesr=onesr,
                  ident=ident, boutb=boutb)
    in_maps = []
    for i in range(N_CORES):
        m = dict(shared)
        m["enc"] = enc_last[i * PC : (i + 1) * PC]
        in_maps.append(m)
    return in_maps


_CACHE = {}


def _run(in_maps, trace=False):
    from concourse.bass_utils import run_bass_kernel_spmd

    if "nc" not in _CACHE:
        _CACHE["nc"] = build_kernel()
    nc = _CACHE["nc"]
    res = run_bass_kernel_spmd(
        nc, in_maps, core_ids=list(range(N_CORES)), trace=trace
    )
    preds = np.concatenate([res.results[i]["out"] for i in range(N_CORES)], axis=0)
    return preds, res


def kernel(encoded_features, step_emb, W_ih0, W_hh0, b_ih0, b_hh0,
           W_ih1, W_hh1, b_ih1, b_hh1, W_out, b_out, num_steps):
    assert int(num_steps) == STEPS
    in_maps = _prep_inputs(encoded_features, step_emb, W_ih0, W_hh0, b_ih0,
                           b_hh0, W_ih1, W_hh1, b_ih1, b_hh1, W_out, b_out)
    preds, _ = _run(in_maps, trace=False)
    return preds


# revision 11
# speedup vs baseline: 1.3474x; 1.3474x over previous
"""Trainium2 Bass kernel for the autoregressive GRU decode head.

Problem: context = mean over zones of encoded_features[:, -1]  -> (B, D)
then 12 autoregressive steps of a 2-layer GRU (H=256) + linear projection
to N=256 zones.  B=1024, data-parallel across 8 NeuronCores (128 batch each).

Structure (per core, feature-major / "transposed" activations):
  actT (128p, 4 slots, 128) bf16 : [h0 c0, h0 c1, h1 c0, h1 c1]
  Gate tensors (PSUM) use layout [p, c*128 + b]; r and z halves of the
  rz gates live in SEPARATE PSUM tiles so the r-sigmoid can fire as soon
  as the r matmuls complete.
  ALL gate biases (incl. the per-step folded step-embedding contribution)
  are injected as K=1 bias-row matmuls into the PSUM accumulation groups,
  so every elementwise chain op is a single full-width instruction:
    sig_r(256) -> t = ghn*r (TT) -> v = gin+t (TT) -> tanh -> q=(z-1)*n
    (STT, all-bf16 SBUF for DVE fast mode) -> hv = c - q  with c = z*h
    computed off the critical path.
  The prediction feedback is folded into layer-0 weights (W_pred@W_out).
  Matmuls for step t+1 that depend only on h0' run during chain1 of step
  t; junk matmul fillers keep the PE p-state at max (the PE halves its
  clock unless continuously busy ~3us).
The encoded_features slice is streamed as bf16 (host-converted); the zone
mean is a pairwise TT-add tree on DVE (bf16 2x levels, f32 tail).
"""

import sys

for _p in ("/opt/trn_rl_repo",):
    if _p not in sys.path:
        sys.path.insert(0, _p)

import numpy as np
import ml_dtypes

import concourse.bass as bass
import concourse.tile as tile
from concourse import mybir
from concourse.vector_clock import ScopedClock

BF16 = ml_dtypes.bfloat16

B, T, NZ, D = 1024, 8, 256, 256
H = 256
STEPS = 12
N_CORES = 8
PC = B // N_CORES  # 128 batch per core

F32 = mybir.dt.float32
BF = mybir.dt.bfloat16
AF = mybir.ActivationFunctionType
OP = mybir.AluOpType

# bias-row slot layout (each slot is 128 wide) in the (1, NBROW*128) bf16
# bias-rows tensor.
_R0R = 0                       # per step: rz0 r rows (2 each)
_R0Z = _R0R + STEPS * 2        # per step: rz0 z rows
_IN0 = _R0Z + STEPS * 2        # per step: in0 rows
_HN0 = _IN0 + STEPS * 2        # 2
_R1R = _HN0 + 2                # 2
_R1Z = _R1R + 2                # 2
_IN1 = _R1Z + 2                # 2
_HN1 = _IN1 + 2                # 2
NBROW = _HN1 + 2

# phase-1 tail warm-up fillers (~213ns each), gated on the 5th enc chunk
NWARM = 12


def _install_tile_drain_patch():
    """walrus (CoreV3) rejects >1 sync wait on the tail drain; spill extras
    onto preceding sync nops."""
    if getattr(tile.TileContext, "_drain_patch_installed", False):
        return

    def _patched(self, tick_clock, wait_clock):
        nc = self.nc
        bb = nc.cur_bb.bb
        drain_bi = nc.sync.drain()
        drain_inst = drain_bi.ins
        wait_clock.add_sem_waits(
            drain_inst, ScopedClock({None: tick_clock.global_clock})
        )
        w = drain_inst.sync_info.on_wait if drain_inst.sync_info else None
        maxw = 1
        if w and len(w) > maxw:
            extra = list(w[maxw:])
            drain_inst.sync_info.on_wait = list(w[:maxw])
            idx = bb.instructions.index(drain_inst)
            nops = []
            for i in range(0, len(extra), maxw):
                nop_bi = nc.sync.nop()
                nop = nop_bi.ins
                si = nop.sync_info
                nop.sync_info = mybir.SyncInfo(
                    on_wait=extra[i : i + maxw],
                    on_update=(si.on_update if si else []),
                )
                bb.instructions.remove(nop)
                nops.append(nop)
            bb.instructions[idx:idx] = nops
        nc.all_engine_barrier()
        popped = nc._tile_sem_poison_stack.pop()
        assert popped is self._sem_poison
        nc.clear_and_free_semaphores(list(self.sems.allocated().values()))
        nc.all_engine_barrier()

    tile.TileContext._drain_and_barrier = _patched
    tile.TileContext._drain_patch_installed = True


def _split_waits(nc, maxw=1):
    """This walrus build rejects instructions carrying more than ~1 sem
    wait; spill extra waits onto same-engine nops placed just before."""
    for bb in nc.main_func.blocks:
        new_list = []
        for inst in bb.instructions:
            si = inst.sync_info
            w = list(si.on_wait) if si and si.on_wait else []
            if len(w) > maxw:
                keep = w[len(w) - maxw:]
                extra = w[: len(w) - maxw]
                si.on_wait = keep
                for i in range(0, len(extra), maxw):
                    nop = mybir.InstNoOp(
                        name=f"{inst.name}-sw{i}", ins=[], outs=[]
                    )
                    nop.engine = inst.engine
                    nop.sync_info = mybir.SyncInfo(
                        on_wait=extra[i : i + maxw], on_update=[]
                    )
                    nc.register_instruction(nop)
                    new_list.append(nop)
            new_list.append(inst)
        bb.instructions[:] = new_list


class _Group:
    """Tracks start/stop flags for a PSUM accumulation group whose matmuls
    are emitted in several program-order batches."""

    def __init__(self, total):
        self.total = total
        self.emitted = 0

    def flags(self):
        start = self.emitted == 0
        self.emitted += 1
        return start, self.emitted == self.total


def build_kernel(nsteps=12):
    """Build the per-core Bass graph (SPMD: same graph on all 8 cores)."""
    _install_tile_drain_patch()
    nc = bass.Bass()

    enc = nc.declare_dram_parameter("enc", [PC, NZ, D], BF, isOutput=False)
    wrz0 = nc.declare_dram_parameter("wrz0", [4, 128, 512], BF, isOutput=False)
    win0 = nc.declare_dram_parameter("win0", [2, 128, 256], BF, isOutput=False)
    whn0 = nc.declare_dram_parameter("whn0", [2, 128, 256], BF, isOutput=False)
    wrz1 = nc.declare_dram_parameter("wrz1", [4, 128, 512], BF, isOutput=False)
    win1 = nc.declare_dram_parameter("win1", [2, 128, 256], BF, isOutput=False)
    whn1 = nc.declare_dram_parameter("whn1", [2, 128, 256], BF, isOutput=False)
    wout = nc.declare_dram_parameter("wout", [2, 128, 256], BF, isOutput=False)
    brows = nc.declare_dram_parameter("brows", [1, NBROW * 128], BF,
                                      isOutput=False)
    onesr = nc.declare_dram_parameter("onesr", [1, 128], BF, isOutput=False)
    ident = nc.declare_dram_parameter("ident", [128, 128], F32, isOutput=False)
    boutb = nc.declare_dram_parameter("boutb", [128, 256], F32, isOutput=False)
    out = nc.declare_dram_parameter("out", [PC, STEPS, NZ], F32, isOutput=True)

    with tile.TileContext(nc) as tc:
        with (
            tc.tile_pool(name="consts", bufs=1) as consts,
            tc.tile_pool(name="state", bufs=1) as state,
            tc.tile_pool(name="enc_pool", bufs=4) as enc_pool,
            tc.tile_pool(name="gates", bufs=2) as gates,
            tc.tile_pool(name="ostage", bufs=2) as ostage,
            tc.tile_pool(name="psum", bufs=1, space="PSUM") as psum,
        ):
            # ---- enc chunk DMAs first: first useful work is the stream ----
            ZCHS = [64, 64, 64, 32, 16, 16]
            NCH = len(ZCHS)
            e_tiles = []
            z0 = 0
            for zch in ZCHS:
                e_sb = enc_pool.tile([128, 64 * D], BF, tag="echunk")
                nc.sync.dma_start(
                    e_sb[:, : zch * D], enc[:, z0 : z0 + zch, :]
                )
                e_tiles.append(e_sb)
                z0 += zch

            # ---- small consts + weights on the scalar HWDGE queue ----
            ones_row = consts.tile([1, 128], BF, tag="ones")
            nc.scalar.dma_start(ones_row[:], onesr[:])
            brow_sb = consts.tile([1, NBROW * 128], BF, tag="brow")
            nc.scalar.dma_start(brow_sb[:], brows[:])
            identity = consts.tile([128, 128], F32, tag="ident")
            nc.scalar.dma_start(identity[:], ident[:])
            w_sb = {}
            for name, ap in (
                ("wrz0", wrz0), ("win0", win0), ("whn0", whn0),
                ("wrz1", wrz1), ("win1", win1), ("whn1", whn1),
                ("wout", wout),
            ):
                kc = ap.shape[0]
                t_ = consts.tile([128, kc, ap.shape[2]], BF, tag=name)
                nc.scalar.dma_start(t_[:], ap.rearrange("k p m -> p k m"))
                w_sb[name] = t_
            boutb_sb = consts.tile([128, 256], F32, tag="boutb")
            nc.scalar.dma_start(boutb_sb[:], boutb[:])

            # prewarm the sigmoid/tanh ACT table during the stream
            warm = consts.tile([128, 1], F32, tag="warm")
            nc.scalar.activation(warm[:], identity[:, 0:1], AF.Sigmoid)

            # ---- phase 1: zone-mean tree per chunk on DVE ----
            partials = state.tile([128, NCH, D], F32, tag="partials")
            for i in range(NCH):
                e_sb = e_tiles[i]
                w = ZCHS[i] * D
                while w > 8 * D:
                    h = w // 2
                    nc.vector.tensor_tensor(
                        e_sb[:, 0:h], e_sb[:, 0:h], e_sb[:, h:w], OP.add
                    )
                    w = h
                # f32 tail: GpSimd for the early chunks (it has slack there);
                # DVE for the late chunks + all combines (shortest exit latency)
                eng = nc.gpsimd if i < 3 else nc.vector
                tmp = gates.tile([128, 4 * D], F32, tag="redtail")
                eng.tensor_tensor(
                    tmp[:], e_sb[:, 0 : 4 * D], e_sb[:, 4 * D : 8 * D], OP.add
                )
                eng.tensor_tensor(
                    tmp[:, 0 : 2 * D], tmp[:, 0 : 2 * D], tmp[:, 2 * D : 4 * D],
                    OP.add,
                )
                eng.tensor_tensor(
                    partials[:, i, :], tmp[:, 0:D], tmp[:, D : 2 * D], OP.add
                )
                if i:
                    nc.vector.tensor_tensor(
                        partials[:, 0, :], partials[:, 0, :], partials[:, i, :],
                        OP.add,
                    )
            ctx = partials[:, 0, :]
            nc.vector.tensor_scalar_mul(ctx, ctx, 1.0 / NZ)

            # ---- PE fillers: dependency-free junk matmuls ----
            junk = psum.tile([128, 512], F32, tag="junk", bufs=1)
            dmy_lhs = w_sb["wrz0"][:, 1, 0:128]
            dmy_rhs = w_sb["wrz0"][:, 2, :]

            def dummies(n):
                for _ in range(n):
                    nc.tensor.matmul(junk[:], dmy_lhs, dmy_rhs,
                                     start=True, stop=True)

            def vgated_dummies(v_, n):
                # top-up fillers gated on the chain's v_ tensor: they run
                # late in the chain window, just before the next gated
                # block, without ever piling up in front of it
                for _ in range(n):
                    nc.tensor.matmul(junk[:, 0:128], identity[:], v_[:, 0:128],
                                     start=True, stop=True)

            def warm_dummies(n, gate_tile):
                # gated on a late enc chunk: ramps the PE p-state up just
                # before the decode phase begins
                for _ in range(n):
                    nc.tensor.matmul(junk[:], dmy_lhs, gate_tile[:, 0:512],
                                     start=True, stop=True)

            warm_dummies(NWARM, e_tiles[4])

            # ---- state: actT slots [h0c0, h0c1, h1c0, h1c1] ----
            # ctx already carries the 1/NZ mean scale
            actT = state.tile([128, 4, 128], BF, tag="actT")
            for c in range(2):
                if c == 0:
                    ctps_t = psum.tile([128, 256], F32, tag="predB", bufs=1)
                    ctps = ctps_t[:, 0:128]
                else:
                    ctps = junk[:, 0:128]
                nc.tensor.transpose(
                    ctps, ctx[:, c * 128 : (c + 1) * 128], identity[:]
                )
                nc.scalar.activation(actT[:, c, :], ctps, AF.Copy)
                nc.vector.tensor_copy(actT[:, 2 + c, :], ctps)
            dummies(5)

            def brow_mm(g, grp, slot, c):
                st, sp = grp.flags()
                nc.tensor.matmul(
                    g[:, c * 128 : (c + 1) * 128],
                    brow_sb[0:1, (slot + c) * 128 : (slot + c + 1) * 128],
                    ones_row[:], start=st, stop=sp,
                )

            def gate_mms(g, grp, w_t, kis, slots, mbase, nch):
                for m in range(nch):
                    ms = slice((mbase + m) * 128, (mbase + m + 1) * 128)
                    gs = slice(m * 128, (m + 1) * 128)
                    for ki, slot in zip(kis, slots):
                        st, sp = grp.flags()
                        nc.tensor.matmul(
                            g[:, gs], w_t[:, ki, ms], actT[:, slot, :],
                            start=st, stop=sp,
                        )

            def chain(layer, g_r, g_z, g_ni):
                h_lo = 0 if layer == 0 else 2
                g_hn = g_ni[:, 0:256]
                g_in = g_ni[:, 256:512]
                s_ = gates.tile([128, 512], BF, tag=f"S{layer}")
                nc.scalar.activation(s_[:, 0:256], g_r[:], AF.Sigmoid)
                nc.scalar.activation(s_[:, 256:512], g_z[:], AF.Sigmoid)
                # s2 = 1 - z, computed exactly as sigmoid(-gz) off-path
                s2 = gates.tile([128, 256], BF, tag=f"S2{layer}")
                nc.scalar.activation(s2[:], g_z[:], AF.Sigmoid, scale=-1.0)
                sr = s_[:, 0:256]
                sz = s_[:, 256:512]
                t_ = gates.tile([128, 256], F32, tag=f"tt{layer}")
                nc.vector.tensor_tensor(t_[:], g_hn, sr, OP.mult)
                v_ = gates.tile([128, 256], F32, tag=f"vv{layer}")
                nc.vector.tensor_tensor(v_[:], g_in, t_[:], OP.add)
                hv = actT[:, h_lo : h_lo + 2, :].rearrange("p a b -> p (a b)")
                c_ = gates.tile([128, 256], BF, tag=f"cc{layer}")
                nc.vector.tensor_tensor(c_[:], sz, hv, OP.mult)
                n_ = gates.tile([128, 256], BF, tag=f"nn{layer}")
                nc.scalar.activation(n_[:], v_[:], AF.Tanh)
                q_ = gates.tile([128, 256], BF, tag=f"qq{layer}")
                nc.vector.tensor_tensor(q_[:], s2[:], n_[:], OP.mult)
                nc.vector.tensor_tensor(hv, c_[:], q_[:], OP.add)
                return s_, v_, n_

            # ---- phase 2: decode steps ----
            # W1(0): step-0 rz0/hn0/in0 h0-part + bias rows (gated on actT)
            g_r0 = psum.tile([128, 256], F32, tag="r0", bufs=1)
            g_z0 = psum.tile([128, 256], F32, tag="z0", bufs=1)
            g_ni0 = psum.tile([128, 512], F32, tag="ni0", bufs=1)
            g_hn0 = g_ni0[:, 0:256]
            g_in0 = g_ni0[:, 256:512]
            gr0 = _Group(6)              # step 0: no feedback
            gz0 = _Group(6)
            gni0 = _Group(8)
            for c in range(2):
                brow_mm(g_r0, gr0, _R0R, c)
            gate_mms(g_r0, gr0, w_sb["wrz0"], (2, 3), (0, 1), 0, 2)
            for c in range(2):
                brow_mm(g_z0, gz0, _R0Z, c)
            gate_mms(g_z0, gz0, w_sb["wrz0"], (2, 3), (0, 1), 2, 2)
            for c in range(2):
                brow_mm(g_hn0, gni0, _HN0, c)
            gate_mms(g_hn0, gni0, w_sb["whn0"], (0, 1), (0, 1), 0, 2)
            for c in range(2):
                brow_mm(g_in0, gni0, _IN0, c)

            prev_pb = None
            for t in range(nsteps):
                # --- block1: pred-feedback matmuls (need h1 of t-1) ---
                if t > 0:
                    gate_mms(g_r0, gr0, w_sb["wrz0"], (0, 1), (2, 3), 0, 2)
                    gate_mms(g_z0, gz0, w_sb["wrz0"], (0, 1), (2, 3), 2, 2)
                    gate_mms(g_in0, gni0, w_sb["win0"], (0, 1), (2, 3), 0, 2)

                dummies(5)

                # --- chain0 ---
                ch0 = chain(0, g_r0, g_z0, g_ni0)

                # --- W2 window (runs on PE during chain0) ---
                g_r1 = psum.tile([128, 256], F32, tag="r1", bufs=1)
                g_z1 = psum.tile([128, 256], F32, tag="z1", bufs=1)
                g_ni1 = psum.tile([128, 512], F32, tag="ni1", bufs=1)
                g_hn1 = g_ni1[:, 0:256]
                g_in1 = g_ni1[:, 256:512]
                gr1 = _Group(10)
                gz1 = _Group(10)
                gni1 = _Group(12)
                for c in range(2):
                    brow_mm(g_r1, gr1, _R1R, c)
                gate_mms(g_r1, gr1, w_sb["wrz1"], (2, 3), (2, 3), 0, 2)
                for c in range(2):
                    brow_mm(g_z1, gz1, _R1Z, c)
                gate_mms(g_z1, gz1, w_sb["wrz1"], (2, 3), (2, 3), 2, 2)
                for c in range(2):
                    brow_mm(g_hn1, gni1, _HN1, c)
                gate_mms(g_hn1, gni1, w_sb["whn1"], (0, 1), (2, 3), 0, 2)
                for c in range(2):
                    brow_mm(g_in1, gni1, _IN1, c)
                # out projection of t-1 (batch-major, off the recurrence)
                if prev_pb is not None:
                    g_pb, pt = prev_pb
                    o_ = ostage.tile([128, 256], F32, tag="ost")
                    nc.vector.tensor_tensor(o_[:], g_pb[:], boutb_sb[:], OP.add)
                    nc.sync.dma_start(out[:, pt, :], o_[:])
                    prev_pb = None
                vgated_dummies(ch0[1], 2)

                # --- block2: layer-1 input-part matmuls (need h0 of t) ---
                gate_mms(g_r1, gr1, w_sb["wrz1"], (0, 1), (0, 1), 0, 2)
                gate_mms(g_z1, gz1, w_sb["wrz1"], (0, 1), (0, 1), 2, 2)
                gate_mms(g_in1, gni1, w_sb["win1"], (0, 1), (0, 1), 0, 2)

                dummies(5)

                # --- chain1 ---
                ch1 = chain(1, g_r1, g_z1, g_ni1)

                # --- W1(t+1) window (runs on PE during chain1) ---
                if t + 1 < nsteps:
                    tn = t + 1
                    g_r0 = psum.tile([128, 256], F32, tag="r0", bufs=1)
                    g_z0 = psum.tile([128, 256], F32, tag="z0", bufs=1)
                    g_ni0 = psum.tile([128, 512], F32, tag="ni0", bufs=1)
                    g_hn0 = g_ni0[:, 0:256]
                    g_in0 = g_ni0[:, 256:512]
                    gr0 = _Group(10)
                    gz0 = _Group(10)
                    gni0 = _Group(12)
                    for c in range(2):
                        brow_mm(g_r0, gr0, _R0R + tn * 2, c)
                    gate_mms(g_r0, gr0, w_sb["wrz0"], (2, 3), (0, 1), 0, 2)
                    for c in range(2):
                        brow_mm(g_z0, gz0, _R0Z + tn * 2, c)
                    gate_mms(g_z0, gz0, w_sb["wrz0"], (2, 3), (0, 1), 2, 2)
                    for c in range(2):
                        brow_mm(g_hn0, gni0, _HN0, c)
                    gate_mms(g_hn0, gni0, w_sb["whn0"], (0, 1), (0, 1), 0, 2)
                    for c in range(2):
                        brow_mm(g_in0, gni0, _IN0 + tn * 2, c)
                # prediction of step t (reads h1' just produced)
                g_pb = psum.tile([128, 256], F32, tag="predB", bufs=1)
                for ki, slot in ((0, 2), (1, 3)):
                    nc.tensor.matmul(
                        g_pb[:], actT[:, slot, :], w_sb["wout"][:, ki, :],
                        start=(ki == 0), stop=(ki == 1),
                    )
                prev_pb = (g_pb, t)
                if t + 1 < nsteps:
                    vgated_dummies(ch1[1], 2)

            # final output write
            g_pb, pt = prev_pb
            o_ = ostage.tile([128, 256], F32, tag="ost")
            nc.vector.tensor_tensor(o_[:], g_pb[:], boutb_sb[:], OP.add)
            nc.sync.dma_start(out[:, pt, :], o_[:])

    _split_waits(nc)
    return nc


def _prep_inputs(encoded_features, step_emb, W_ih0, W_hh0, b_ih0, b_hh0,
                 W_ih1, W_hh1, b_ih1, b_hh1, W_out, b_out):
    """Host-side: slice/shard the big input, transpose + cast weights,
    fold the output projection into layer-0 input weights, fold the
    step-embedding matmul + all additive constants into bias rows."""
    f4 = np.float32
    enc_last = np.asarray(encoded_features)[:, -1].astype(BF16)
    enc_last = np.ascontiguousarray(enc_last)

    W_ih0 = np.asarray(W_ih0, f4)
    W_hh0 = np.asarray(W_hh0, f4)
    W_ih1 = np.asarray(W_ih1, f4)
    W_hh1 = np.asarray(W_hh1, f4)
    W_out = np.asarray(W_out, f4)
    step_emb = np.asarray(step_emb, f4)
    b_ih0 = np.asarray(b_ih0, f4)
    b_hh0 = np.asarray(b_hh0, f4)
    b_ih1 = np.asarray(b_ih1, f4)
    b_hh1 = np.asarray(b_hh1, f4)
    b_out = np.asarray(b_out, f4)

    W_emb = W_ih0[:, :D]          # (768, 256)
    W_pred = W_ih0[:, D:]         # (768, 256)
    W_fold = W_pred @ W_out       # (768, 256): pred feedback folded onto h1
    b_fold = W_pred @ b_out       # (768,)

    # gi_emb[t] = W_emb @ step_emb[t] + b_ih0  -> (12, 768)
    gi_emb = step_emb[:STEPS] @ W_emb.T + b_ih0[None, :]

    def kstack(*mats_cols):
        chunks = []
        for mat, cols in mats_cols:
            mt = np.ascontiguousarray(mat.T[:, cols])  # (K, M)
            for k in range(0, mt.shape[0], 128):
                chunks.append(mt[k : k + 128])
        return np.stack(chunks).astype(BF16)  # (nk, 128, M)

    rz = slice(0, 512)
    ng = slice(512, 768)
    wrz0 = kstack((W_fold, rz), (W_hh0, rz))          # K: h1c0,h1c1,h0c0,h0c1
    win0 = kstack((W_fold, ng))
    whn0 = kstack((W_hh0, ng))
    wrz1 = kstack((W_ih1, rz), (W_hh1, rz))           # K: h0c0,h0c1,h1c0,h1c1
    win1 = kstack((W_ih1, ng))
    whn1 = kstack((W_hh1, ng))
    wout = np.stack([np.ascontiguousarray(W_out.T)[k : k + 128] for k in (0, 128)]
                    ).astype(BF16)                    # (2, 128, 256)

    brows = np.zeros(NBROW * 128, f4)

    def put(base, vec):
        brows[base * 128 : base * 128 + len(vec)] = vec

    for t in range(STEPS):
        extra = b_fold if t > 0 else np.zeros_like(b_fold)
        rzrow = gi_emb[t, :512] + b_hh0[:512] + extra[:512]
        put(_R0R + t * 2, rzrow[:256])
        put(_R0Z + t * 2, rzrow[256:])
        put(_IN0 + t * 2, gi_emb[t, 512:] + extra[512:])
    put(_HN0, b_hh0[512:])
    rz1row = b_ih1[:512] + b_hh1[:512]
    put(_R1R, rz1row[:256])
    put(_R1Z, rz1row[256:])
    put(_IN1, b_ih1[512:])
    put(_HN1, b_hh1[512:])
    brows = brows.astype(BF16)[None, :]

    onesr = np.ones((1, 128), BF16)
    ident = np.eye(128, dtype=f4)
    boutb = np.broadcast_to(b_out[None, :], (128, 256)).astype(f4).copy()

    shared = dict(wrz0=wrz0, win0=win0, whn0=whn0, wrz1=wrz1, win1=win1,
                  whn1=whn1, wout=wout, brows=brows, onesr=onesr,
                  ident=ident, boutb=boutb)
    in_maps = []
    for i in range(N_CORES):
        m = dict(shared)
        m["enc"] = enc_last[i * PC : (i + 1) * PC]
        in_maps.append(m)
    return in_maps


_CACHE = {}


def _run(in_maps, trace=False):
    from concourse.bass_utils import run_bass_kernel_spmd

    if "nc" not in _CACHE:
        _CACHE["nc"] = build_kernel()
    nc = _CACHE["nc"]
    res = run_bass_kernel_spmd(
        nc, in_maps, core_ids=list(range(N_CORES)), trace=trace
    )
    preds = np.concatenate([res.results[i]["out"] for i in range(N_CORES)], axis=0)
    return preds, res


def kernel(encoded_features, step_emb, W_ih0, W_hh0, b_ih0, b_hh0,
           W_ih1, W_hh1, b_ih1, b_hh1, W_out, b_out, num_steps):
    assert int(num_steps) == STEPS
    in_maps = _prep_inputs(encoded_features, step_emb, W_ih0, W_hh0, b_ih0,
                           b_hh0, W_ih1, W_hh1, b_ih1, b_hh1, W_out, b_out)
    preds, _ = _run(in_maps, trace=False)
    return preds


# revision 12
# speedup vs baseline: 1.5023x; 1.1150x over previous
"""Trainium2 Bass kernel for the autoregressive GRU decode head.

Problem: context = mean over zones of encoded_features[:, -1]  -> (B, D)
then 12 autoregressive steps of a 2-layer GRU (H=256) + linear projection
to N=256 zones.  B=1024, data-parallel across 8 NeuronCores (128 batch each).

Structure (per core, feature-major / "transposed" activations):
  actT (128p, 4 slots, 128) bf16 : [h0 c0, h0 c1, h1 c0, h1 c1]
  Gate tensors (PSUM) use layout [p, c*128 + b]; r and z halves of the
  rz gates live in SEPARATE PSUM tiles so the r-sigmoid can fire as soon
  as the r matmuls complete.
  ALL gate biases (incl. the per-step folded step-embedding contribution)
  are injected as K=1 bias-row matmuls into the PSUM accumulation groups,
  so every elementwise chain op is a single full-width instruction:
    sig_r(256) -> t = ghn*r (TT) -> v = gin+t (TT) -> tanh -> q=(z-1)*n
    (STT, all-bf16 SBUF for DVE fast mode) -> hv = c - q  with c = z*h
    computed off the critical path.
  The prediction feedback is folded into layer-0 weights (W_pred@W_out).
  Matmuls for step t+1 that depend only on h0' run during chain1 of step
  t; junk matmul fillers keep the PE p-state at max (the PE halves its
  clock unless continuously busy ~3us).
The encoded_features slice is streamed as bf16 (host-converted); the zone
mean is a pairwise TT-add tree on DVE (bf16 2x levels, f32 tail).
"""

import sys

for _p in ("/opt/trn_rl_repo",):
    if _p not in sys.path:
        sys.path.insert(0, _p)

import numpy as np
import ml_dtypes

import concourse.bass as bass
import concourse.tile as tile
from concourse import mybir
from concourse.vector_clock import ScopedClock

BF16 = ml_dtypes.bfloat16

B, T, NZ, D = 1024, 8, 256, 256
H = 256
STEPS = 12
N_CORES = 8
PC = B // N_CORES  # 128 batch per core

F32 = mybir.dt.float32
BF = mybir.dt.bfloat16
AF = mybir.ActivationFunctionType
OP = mybir.AluOpType

# bias-row slot layout (each slot is 128 wide) in the (1, NBROW*128) bf16
# bias-rows tensor.
_R0R = 0                       # per step: rz0 r rows (2 each)
_R0Z = _R0R + STEPS * 2        # per step: rz0 z rows
_IN0 = _R0Z + STEPS * 2        # per step: in0 rows
_HN0 = _IN0 + STEPS * 2        # 2
_R1R = _HN0 + 2                # 2
_R1Z = _R1R + 2                # 2
_IN1 = _R1Z + 2                # 2
_HN1 = _IN1 + 2                # 2
NBROW = _HN1 + 2

# phase-1 tail warm-up fillers (~213ns each), gated on the 5th enc chunk
NWARM = 12


def _install_tile_drain_patch():
    """walrus (CoreV3) rejects >1 sync wait on the tail drain; spill extras
    onto preceding sync nops."""
    if getattr(tile.TileContext, "_drain_patch_installed", False):
        return

    def _patched(self, tick_clock, wait_clock):
        nc = self.nc
        bb = nc.cur_bb.bb
        drain_bi = nc.sync.drain()
        drain_inst = drain_bi.ins
        wait_clock.add_sem_waits(
            drain_inst, ScopedClock({None: tick_clock.global_clock})
        )
        w = drain_inst.sync_info.on_wait if drain_inst.sync_info else None
        maxw = 1
        if w and len(w) > maxw:
            extra = list(w[maxw:])
            drain_inst.sync_info.on_wait = list(w[:maxw])
            idx = bb.instructions.index(drain_inst)
            nops = []
            for i in range(0, len(extra), maxw):
                nop_bi = nc.sync.nop()
                nop = nop_bi.ins
                si = nop.sync_info
                nop.sync_info = mybir.SyncInfo(
                    on_wait=extra[i : i + maxw],
                    on_update=(si.on_update if si else []),
                )
                bb.instructions.remove(nop)
                nops.append(nop)
            bb.instructions[idx:idx] = nops
        nc.all_engine_barrier()
        popped = nc._tile_sem_poison_stack.pop()
        assert popped is self._sem_poison
        nc.clear_and_free_semaphores(list(self.sems.allocated().values()))
        nc.all_engine_barrier()

    tile.TileContext._drain_and_barrier = _patched
    tile.TileContext._drain_patch_installed = True


def _split_waits(nc, maxw=1):
    """This walrus build rejects instructions carrying more than ~1 sem
    wait; spill extra waits onto same-engine nops placed just before."""
    for bb in nc.main_func.blocks:
        new_list = []
        for inst in bb.instructions:
            si = inst.sync_info
            w = list(si.on_wait) if si and si.on_wait else []
            if len(w) > maxw:
                keep = w[len(w) - maxw:]
                extra = w[: len(w) - maxw]
                si.on_wait = keep
                for i in range(0, len(extra), maxw):
                    nop = mybir.InstNoOp(
                        name=f"{inst.name}-sw{i}", ins=[], outs=[]
                    )
                    nop.engine = inst.engine
                    nop.sync_info = mybir.SyncInfo(
                        on_wait=extra[i : i + maxw], on_update=[]
                    )
                    nc.register_instruction(nop)
                    new_list.append(nop)
            new_list.append(inst)
        bb.instructions[:] = new_list


class _Group:
    """Tracks start/stop flags for a PSUM accumulation group whose matmuls
    are emitted in several program-order batches."""

    def __init__(self, total):
        self.total = total
        self.emitted = 0

    def flags(self):
        start = self.emitted == 0
        self.emitted += 1
        return start, self.emitted == self.total


def build_kernel(nsteps=12):
    """Build the per-core Bass graph (SPMD: same graph on all 8 cores)."""
    _install_tile_drain_patch()
    nc = bass.Bass()

    enc = nc.declare_dram_parameter("enc", [PC, NZ, D], BF, isOutput=False)
    wrz0 = nc.declare_dram_parameter("wrz0", [4, 128, 512], BF, isOutput=False)
    win0 = nc.declare_dram_parameter("win0", [2, 128, 256], BF, isOutput=False)
    whn0 = nc.declare_dram_parameter("whn0", [2, 128, 256], BF, isOutput=False)
    wrz1 = nc.declare_dram_parameter("wrz1", [4, 128, 512], BF, isOutput=False)
    win1 = nc.declare_dram_parameter("win1", [2, 128, 256], BF, isOutput=False)
    whn1 = nc.declare_dram_parameter("whn1", [2, 128, 256], BF, isOutput=False)
    wout = nc.declare_dram_parameter("wout", [2, 128, 256], BF, isOutput=False)
    brows = nc.declare_dram_parameter("brows", [1, NBROW * 128], BF,
                                      isOutput=False)
    onesr = nc.declare_dram_parameter("onesr", [1, 128], BF, isOutput=False)
    ident = nc.declare_dram_parameter("ident", [128, 128], F32, isOutput=False)
    boutb = nc.declare_dram_parameter("boutb", [128, 256], F32, isOutput=False)
    out = nc.declare_dram_parameter("out", [PC, STEPS, NZ], F32, isOutput=True)

    with tile.TileContext(nc) as tc:
        with (
            tc.tile_pool(name="consts", bufs=1) as consts,
            tc.tile_pool(name="state", bufs=1) as state,
            tc.tile_pool(name="enc_pool", bufs=4) as enc_pool,
            tc.tile_pool(name="gates", bufs=2) as gates,
            tc.tile_pool(name="ostage", bufs=2) as ostage,
            tc.tile_pool(name="psum", bufs=1, space="PSUM") as psum,
        ):
            # ---- enc chunk DMAs first: first useful work is the stream ----
            ZCHS = [64, 64, 64, 32, 16, 16]
            NCH = len(ZCHS)
            e_tiles = []
            z0 = 0
            for zch in ZCHS:
                e_sb = enc_pool.tile([128, 64 * D], BF, tag="echunk")
                nc.sync.dma_start(
                    e_sb[:, : zch * D], enc[:, z0 : z0 + zch, :]
                )
                e_tiles.append(e_sb)
                z0 += zch

            # ---- small consts + weights on the scalar HWDGE queue ----
            ones_row = consts.tile([1, 128], BF, tag="ones")
            nc.scalar.dma_start(ones_row[:], onesr[:])
            brow_sb = consts.tile([1, NBROW * 128], BF, tag="brow")
            nc.scalar.dma_start(brow_sb[:], brows[:])
            identity = consts.tile([128, 128], F32, tag="ident")
            nc.scalar.dma_start(identity[:], ident[:])
            w_sb = {}
            for name, ap in (
                ("wrz0", wrz0), ("win0", win0), ("whn0", whn0),
                ("wrz1", wrz1), ("win1", win1), ("whn1", whn1),
                ("wout", wout),
            ):
                kc = ap.shape[0]
                t_ = consts.tile([128, kc, ap.shape[2]], BF, tag=name)
                nc.scalar.dma_start(t_[:], ap.rearrange("k p m -> p k m"))
                w_sb[name] = t_
            boutb_sb = consts.tile([128, 256], F32, tag="boutb")
            nc.scalar.dma_start(boutb_sb[:], boutb[:])

            # prewarm the sigmoid/tanh ACT table during the stream
            warm = consts.tile([128, 1], F32, tag="warm")
            nc.scalar.activation(warm[:], identity[:, 0:1], AF.Sigmoid)

            # ---- phase 1: zone-mean tree per chunk on DVE ----
            partials = state.tile([128, NCH, D], F32, tag="partials")
            for i in range(NCH):
                e_sb = e_tiles[i]
                w = ZCHS[i] * D
                while w > 8 * D:
                    h = w // 2
                    nc.vector.tensor_tensor(
                        e_sb[:, 0:h], e_sb[:, 0:h], e_sb[:, h:w], OP.add
                    )
                    w = h
                # f32 tail: GpSimd for the early chunks (it has slack there);
                # DVE for the late chunks + all combines (shortest exit latency)
                eng = nc.gpsimd if i < 3 else nc.vector
                tmp = gates.tile([128, 4 * D], F32, tag="redtail")
                eng.tensor_tensor(
                    tmp[:], e_sb[:, 0 : 4 * D], e_sb[:, 4 * D : 8 * D], OP.add
                )
                eng.tensor_tensor(
                    tmp[:, 0 : 2 * D], tmp[:, 0 : 2 * D], tmp[:, 2 * D : 4 * D],
                    OP.add,
                )
                eng.tensor_tensor(
                    partials[:, i, :], tmp[:, 0:D], tmp[:, D : 2 * D], OP.add
                )
                # per-engine accumulators: Pool sums chunks 0-2 into
                # partials[0], DVE sums chunks 3-5 into partials[3]; a
                # single cross-engine merge happens at the very end
                if i in (1, 2):
                    nc.gpsimd.tensor_tensor(
                        partials[:, 0, :], partials[:, 0, :], partials[:, i, :],
                        OP.add,
                    )
                elif i in (4, 5):
                    nc.vector.tensor_tensor(
                        partials[:, 3, :], partials[:, 3, :], partials[:, i, :],
                        OP.add,
                    )
            ctx = partials[:, 0, :]
            nc.vector.tensor_tensor(ctx, ctx, partials[:, 3, :], OP.add)
            nc.vector.tensor_scalar_mul(ctx, ctx, 1.0 / NZ)

            # ---- PE fillers: dependency-free junk matmuls ----
            junk = psum.tile([128, 512], F32, tag="junk", bufs=1)
            dmy_lhs = w_sb["wrz0"][:, 1, 0:128]
            dmy_rhs = w_sb["wrz0"][:, 2, :]

            def dummies(n):
                for _ in range(n):
                    nc.tensor.matmul(junk[:], dmy_lhs, dmy_rhs,
                                     start=True, stop=True)

            def vgated_dummies(v_, n):
                # top-up fillers gated on the chain's v_ tensor: they run
                # late in the chain window, just before the next gated
                # block, without ever piling up in front of it
                for _ in range(n):
                    nc.tensor.matmul(junk[:, 0:128], identity[:], v_[:, 0:128],
                                     start=True, stop=True)

            def warm_dummies(n, gate_tile):
                # gated on a late enc chunk: ramps the PE p-state up just
                # before the decode phase begins
                for _ in range(n):
                    nc.tensor.matmul(junk[:], dmy_lhs, gate_tile[:, 0:512],
                                     start=True, stop=True)

            warm_dummies(NWARM, e_tiles[4])

            # ---- state: actT slots [h0c0, h0c1, h1c0, h1c1] ----
            # ctx already carries the 1/NZ mean scale
            actT = state.tile([128, 4, 128], BF, tag="actT")
            for c in range(2):
                if c == 0:
                    ctps_t = psum.tile([128, 256], F32, tag="predB", bufs=1)
                    ctps = ctps_t[:, 0:128]
                else:
                    ctps = junk[:, 0:128]
                nc.tensor.transpose(
                    ctps, ctx[:, c * 128 : (c + 1) * 128], identity[:]
                )
                nc.scalar.activation(actT[:, c, :], ctps, AF.Copy)
                nc.vector.tensor_copy(actT[:, 2 + c, :], ctps)
            dummies(5)

            def brow_mm(g, grp, slot, c):
                st, sp = grp.flags()
                nc.tensor.matmul(
                    g[:, c * 128 : (c + 1) * 128],
                    brow_sb[0:1, (slot + c) * 128 : (slot + c + 1) * 128],
                    ones_row[:], start=st, stop=sp,
                )

            def gate_mms(g, grp, w_t, kis, slots, mbase, nch):
                for m in range(nch):
                    ms = slice((mbase + m) * 128, (mbase + m + 1) * 128)
                    gs = slice(m * 128, (m + 1) * 128)
                    for ki, slot in zip(kis, slots):
                        st, sp = grp.flags()
                        nc.tensor.matmul(
                            g[:, gs], w_t[:, ki, ms], actT[:, slot, :],
                            start=st, stop=sp,
                        )

            def chain(layer, g_r, g_z, g_ni):
                h_lo = 0 if layer == 0 else 2
                g_hn = g_ni[:, 0:256]
                g_in = g_ni[:, 256:512]
                s_ = gates.tile([128, 512], BF, tag=f"S{layer}")
                nc.scalar.activation(s_[:, 0:256], g_r[:], AF.Sigmoid)
                nc.scalar.activation(s_[:, 256:512], g_z[:], AF.Sigmoid)
                # s2 = 1 - z, computed exactly as sigmoid(-gz) off-path
                s2 = gates.tile([128, 256], BF, tag=f"S2{layer}")
                nc.scalar.activation(s2[:], g_z[:], AF.Sigmoid, scale=-1.0)
                sr = s_[:, 0:256]
                sz = s_[:, 256:512]
                t_ = gates.tile([128, 256], F32, tag=f"tt{layer}")
                nc.vector.tensor_tensor(t_[:], g_hn, sr, OP.mult)
                v_ = gates.tile([128, 256], F32, tag=f"vv{layer}")
                nc.vector.tensor_tensor(v_[:], g_in, t_[:], OP.add)
                hv = actT[:, h_lo : h_lo + 2, :].rearrange("p a b -> p (a b)")
                c_ = gates.tile([128, 256], BF, tag=f"cc{layer}")
                nc.vector.tensor_tensor(c_[:], sz, hv, OP.mult)
                n_ = gates.tile([128, 256], BF, tag=f"nn{layer}")
                nc.scalar.activation(n_[:], v_[:], AF.Tanh)
                q_ = gates.tile([128, 256], BF, tag=f"qq{layer}")
                nc.vector.tensor_tensor(q_[:], s2[:], n_[:], OP.mult)
                nc.vector.tensor_tensor(hv, c_[:], q_[:], OP.add)
                return s_, v_, n_

            # ---- phase 2: decode steps ----
            # W1(0): step-0 rz0/hn0/in0 h0-part + bias rows (gated on actT)
            g_r0 = psum.tile([128, 256], F32, tag="r0", bufs=1)
            g_z0 = psum.tile([128, 256], F32, tag="z0", bufs=1)
            g_ni0 = psum.tile([128, 512], F32, tag="ni0", bufs=1)
            g_hn0 = g_ni0[:, 0:256]
            g_in0 = g_ni0[:, 256:512]
            gr0 = _Group(6)              # step 0: no feedback
            gz0 = _Group(6)
            gni0 = _Group(8)
            for c in range(2):
                brow_mm(g_r0, gr0, _R0R, c)
            gate_mms(g_r0, gr0, w_sb["wrz0"], (2, 3), (0, 1), 0, 2)
            for c in range(2):
                brow_mm(g_z0, gz0, _R0Z, c)
            gate_mms(g_z0, gz0, w_sb["wrz0"], (2, 3), (0, 1), 2, 2)
            for c in range(2):
                brow_mm(g_hn0, gni0, _HN0, c)
            gate_mms(g_hn0, gni0, w_sb["whn0"], (0, 1), (0, 1), 0, 2)
            for c in range(2):
                brow_mm(g_in0, gni0, _IN0, c)

            prev_pb = None
            for t in range(nsteps):
                # --- block1: pred-feedback matmuls (need h1 of t-1) ---
                if t > 0:
                    gate_mms(g_r0, gr0, w_sb["wrz0"], (0, 1), (2, 3), 0, 2)
                    gate_mms(g_z0, gz0, w_sb["wrz0"], (0, 1), (2, 3), 2, 2)
                    gate_mms(g_in0, gni0, w_sb["win0"], (0, 1), (2, 3), 0, 2)

                dummies(3)

                # --- chain0 ---
                ch0 = chain(0, g_r0, g_z0, g_ni0)

                # --- W2 window (runs on PE during chain0) ---
                g_r1 = psum.tile([128, 256], F32, tag="r1", bufs=1)
                g_z1 = psum.tile([128, 256], F32, tag="z1", bufs=1)
                g_ni1 = psum.tile([128, 512], F32, tag="ni1", bufs=1)
                g_hn1 = g_ni1[:, 0:256]
                g_in1 = g_ni1[:, 256:512]
                gr1 = _Group(10)
                gz1 = _Group(10)
                gni1 = _Group(12)
                for c in range(2):
                    brow_mm(g_r1, gr1, _R1R, c)
                gate_mms(g_r1, gr1, w_sb["wrz1"], (2, 3), (2, 3), 0, 2)
                for c in range(2):
                    brow_mm(g_z1, gz1, _R1Z, c)
                gate_mms(g_z1, gz1, w_sb["wrz1"], (2, 3), (2, 3), 2, 2)
                for c in range(2):
                    brow_mm(g_hn1, gni1, _HN1, c)
                gate_mms(g_hn1, gni1, w_sb["whn1"], (0, 1), (2, 3), 0, 2)
                for c in range(2):
                    brow_mm(g_in1, gni1, _IN1, c)
                # out projection of t-1 (batch-major, off the recurrence)
                if prev_pb is not None:
                    g_pb, pt = prev_pb
                    o_ = ostage.tile([128, 256], F32, tag="ost")
                    nc.vector.tensor_tensor(o_[:], g_pb[:], boutb_sb[:], OP.add)
                    nc.sync.dma_start(out[:, pt, :], o_[:])
                    prev_pb = None
                vgated_dummies(ch0[1], 1)

                # --- block2: layer-1 input-part matmuls (need h0 of t) ---
                gate_mms(g_r1, gr1, w_sb["wrz1"], (0, 1), (0, 1), 0, 2)
                gate_mms(g_z1, gz1, w_sb["wrz1"], (0, 1), (0, 1), 2, 2)
                gate_mms(g_in1, gni1, w_sb["win1"], (0, 1), (0, 1), 0, 2)

                dummies(3)

                # --- chain1 ---
                ch1 = chain(1, g_r1, g_z1, g_ni1)

                # --- W1(t+1) window (runs on PE during chain1) ---
                if t + 1 < nsteps:
                    tn = t + 1
                    g_r0 = psum.tile([128, 256], F32, tag="r0", bufs=1)
                    g_z0 = psum.tile([128, 256], F32, tag="z0", bufs=1)
                    g_ni0 = psum.tile([128, 512], F32, tag="ni0", bufs=1)
                    g_hn0 = g_ni0[:, 0:256]
                    g_in0 = g_ni0[:, 256:512]
                    gr0 = _Group(10)
                    gz0 = _Group(10)
                    gni0 = _Group(12)
                    for c in range(2):
                        brow_mm(g_r0, gr0, _R0R + tn * 2, c)
                    gate_mms(g_r0, gr0, w_sb["wrz0"], (2, 3), (0, 1), 0, 2)
                    for c in range(2):
                        brow_mm(g_z0, gz0, _R0Z + tn * 2, c)
                    gate_mms(g_z0, gz0, w_sb["wrz0"], (2, 3), (0, 1), 2, 2)
                    for c in range(2):
                        brow_mm(g_hn0, gni0, _HN0, c)
                    gate_mms(g_hn0, gni0, w_sb["whn0"], (0, 1), (0, 1), 0, 2)
                    for c in range(2):
                        brow_mm(g_in0, gni0, _IN0 + tn * 2, c)
                # prediction of step t (reads h1' just produced)
                g_pb = psum.tile([128, 256], F32, tag="predB", bufs=1)
                for ki, slot in ((0, 2), (1, 3)):
                    nc.tensor.matmul(
                        g_pb[:], actT[:, slot, :], w_sb["wout"][:, ki, :],
                        start=(ki == 0), stop=(ki == 1),
                    )
                prev_pb = (g_pb, t)
                if t + 1 < nsteps:
                    vgated_dummies(ch1[1], 1)

            # final output write
            g_pb, pt = prev_pb
            o_ = ostage.tile([128, 256], F32, tag="ost")
            nc.vector.tensor_tensor(o_[:], g_pb[:], boutb_sb[:], OP.add)
            nc.sync.dma_start(out[:, pt, :], o_[:])

    _split_waits(nc)
    return nc


def _prep_inputs(encoded_features, step_emb, W_ih0, W_hh0, b_ih0, b_hh0,
                 W_ih1, W_hh1, b_ih1, b_hh1, W_out, b_out):
    """Host-side: slice/shard the big input, transpose + cast weights,
    fold the output projection into layer-0 input weights, fold the
    step-embedding matmul + all additive constants into bias rows."""
    f4 = np.float32
    enc_last = np.asarray(encoded_features)[:, -1].astype(BF16)
    enc_last = np.ascontiguousarray(enc_last)

    W_ih0 = np.asarray(W_ih0, f4)
    W_hh0 = np.asarray(W_hh0, f4)
    W_ih1 = np.asarray(W_ih1, f4)
    W_hh1 = np.asarray(W_hh1, f4)
    W_out = np.asarray(W_out, f4)
    step_emb = np.asarray(step_emb, f4)
    b_ih0 = np.asarray(b_ih0, f4)
    b_hh0 = np.asarray(b_hh0, f4)
    b_ih1 = np.asarray(b_ih1, f4)
    b_hh1 = np.asarray(b_hh1, f4)
    b_out = np.asarray(b_out, f4)

    W_emb = W_ih0[:, :D]          # (768, 256)
    W_pred = W_ih0[:, D:]         # (768, 256)
    W_fold = W_pred @ W_out       # (768, 256): pred feedback folded onto h1
    b_fold = W_pred @ b_out       # (768,)

    # gi_emb[t] = W_emb @ step_emb[t] + b_ih0  -> (12, 768)
    gi_emb = step_emb[:STEPS] @ W_emb.T + b_ih0[None, :]

    def kstack(*mats_cols):
        chunks = []
        for mat, cols in mats_cols:
            mt = np.ascontiguousarray(mat.T[:, cols])  # (K, M)
            for k in range(0, mt.shape[0], 128):
                chunks.append(mt[k : k + 128])
        return np.stack(chunks).astype(BF16)  # (nk, 128, M)

    rz = slice(0, 512)
    ng = slice(512, 768)
    wrz0 = kstack((W_fold, rz), (W_hh0, rz))          # K: h1c0,h1c1,h0c0,h0c1
    win0 = kstack((W_fold, ng))
    whn0 = kstack((W_hh0, ng))
    wrz1 = kstack((W_ih1, rz), (W_hh1, rz))           # K: h0c0,h0c1,h1c0,h1c1
    win1 = kstack((W_ih1, ng))
    whn1 = kstack((W_hh1, ng))
    wout = np.stack([np.ascontiguousarray(W_out.T)[k : k + 128] for k in (0, 128)]
                    ).astype(BF16)                    # (2, 128, 256)

    brows = np.zeros(NBROW * 128, f4)

    def put(base, vec):
        brows[base * 128 : base * 128 + len(vec)] = vec

    for t in range(STEPS):
        extra = b_fold if t > 0 else np.zeros_like(b_fold)
        rzrow = gi_emb[t, :512] + b_hh0[:512] + extra[:512]
        put(_R0R + t * 2, rzrow[:256])
        put(_R0Z + t * 2, rzrow[256:])
        put(_IN0 + t * 2, gi_emb[t, 512:] + extra[512:])
    put(_HN0, b_hh0[512:])
    rz1row = b_ih1[:512] + b_hh1[:512]
    put(_R1R, rz1row[:256])
    put(_R1Z, rz1row[256:])
    put(_IN1, b_ih1[512:])
    put(_HN1, b_hh1[512:])
    brows = brows.astype(BF16)[None, :]

    onesr = np.ones((1, 128), BF16)
    ident = np.eye(128, dtype=f4)
    boutb = np.broadcast_to(b_out[None, :], (128, 256)).astype(f4).copy()

    shared = dict(wrz0=wrz0, win0=win0, whn0=whn0, wrz1=wrz1, win1=win1,
                  whn1=whn1, wout=wout, brows=brows, onesr=onesr,
                  ident=ident, boutb=boutb)
    in_maps = []
    for i in range(N_CORES):
        m = dict(shared)
        m["enc"] = enc_last[i * PC : (i + 1) * PC]
        in_maps.append(m)
    return in_maps


_CACHE = {}


def _run(in_maps, trace=False):
    from concourse.bass_utils import run_bass_kernel_spmd

    if "nc" not in _CACHE:
        _CACHE["nc"] = build_kernel()
    nc = _CACHE["nc"]
    res = run_bass_kernel_spmd(
        nc, in_maps, core_ids=list(range(N_CORES)), trace=trace
    )
    preds = np.concatenate([res.results[i]["out"] for i in range(N_CORES)], axis=0)
    return preds, res


def kernel(encoded_features, step_emb, W_ih0, W_hh0, b_ih0, b_hh0,
           W_ih1, W_hh1, b_ih1, b_hh1, W_out, b_out, num_steps):
    assert int(num_steps) == STEPS
    in_maps = _prep_inputs(encoded_features, step_emb, W_ih0, W_hh0, b_ih0,
                           b_hh0, W_ih1, W_hh1, b_ih1, b_hh1, W_out, b_out)
    preds, _ = _run(in_maps, trace=False)
    return preds
